# revision 26
# baseline (speedup 1.0000x reference)
"""Trainium2 Bass kernel: NKQuantizer2 top-k masking (k=8).

reference:  kh = topk_hot(x, 8)          # [B,S,Q] 0/1 mask, top-8 per token
            out = einsum('bsq,eq->bse', kh, W)

Per token: out[t] = sum_{q in top8(x[t])} W[:, q] -- an 8-way embedding
gather-sum from W.T [Q, E].

Strategy (data-parallel over tokens across 8 cores, W.T bf16 in HBM):
  Per 128-token tile on each core:
    1. DMA x tile [128, 8192] f32 HBM->SBUF (HWDGE, 8 loads = the 8 HW
       lanes, one wait each)
    2. DVE Max8 -> top-8 values per token; DVE MaxIndex -> their indices
       (exact, ties -> first occurrence, matching jax.lax.top_k)
    3. ONE batched indirect-DMA gather per tile (SWDGE):
       g[p, j, :] = WT[idx8[p, j], :], 1024 descriptors in a single
       instruction -- amortizes the ~1us fixed SWDGE overhead that
       dominated when issued as 8 separate accumulate-gathers, and drops
       the CCE read-modify-write from the DMA datapath.
    4. DVE tree-reduce over j (bf16 pass, then f32) -> o [128, 512] f32
    5. Store o -> out rows, also on the SWDGE FIFO.

Toolchain constraints handled here:
  * Any instruction can encode at most ONE semaphore wait; waits on the
    same semaphore merge (max tick), so multiple deps are fine only if
    they land on one engine's semaphore.
  * 8 global HWDGE lanes + 8 SWDGE lanes; a DMA on a reused lane gets a
    mandatory ring wait injected by codegen, which uses up its one slot.
    So SWDGE ops 9..16 carry their cross-engine dep on a preceding Pool
    nop shim instead (the SWDGE descriptor generator executes waits in
    program order, so a nop wait gates the following descriptor).
  * The tile scheduler reorders instruction streams, which can turn
    same-engine deps into semaphore waits (and scrambles walrus's
    ring-lane assignment). Every instruction is chained to its
    same-engine predecessor with a non-sync edge to pin stream order.
"""

import numpy as np
import ml_dtypes

import concourse.bass as bass
import concourse.mybir as mybir
import concourse.tile as tile
from concourse.bass_utils import run_bass_kernel_spmd
from concourse.tile_rust import add_dep_helper

B, S, Q, E, TOPK = 4, 2048, 8192, 512, 8
N_CORES = 8
P = 128
T_TOTAL = B * S                 # 8192 tokens
T_CORE = T_TOTAL // N_CORES     # 1024 tokens per core

F32 = mybir.dt.float32
BF16 = mybir.dt.bfloat16
U32 = mybir.dt.uint32


def build_bass(t_core=T_CORE, q=Q, e=E):
    """Build the per-core Bass program (SPMD: same program on all cores)."""
    n_tiles = t_core // P
    xbufs = min(4, n_tiles)
    gbufs = min(3, n_tiles)

    nc = bass.Bass(trn_type="TRN2", target_bir_lowering=False)
    x_d = nc.dram_tensor("x", [t_core, q], F32, kind="ExternalInput")
    wt_d = nc.dram_tensor("wt", [q, e], BF16, kind="ExternalInput")
    out_d = nc.dram_tensor("out", [t_core, e], F32, kind="ExternalOutput")

    created = {}         # name -> mybir instruction, everything we emit
    stream_last = {}     # engine-stream key -> last instruction (pinning)

    def emit(key, bass_ins):
        """Register an instruction and chain it into its engine stream."""
        ins = bass_ins.ins
        if key in stream_last:
            add_dep_helper(ins, stream_last[key], False, f"{key} order")
        stream_last[key] = ins
        created[ins.name] = ins
        return bass_ins

    n_swdge = 0          # SWDGE FIFO slot counter (8 lanes before reuse)
    swdge_fifo = []      # all SWDGE DMAs in program order

    def swdge_emit(emit_fn):
        """Emit a SWDGE DMA. In the first 8 FIFO slots its (merged,
        single-semaphore) cross-engine deps ride on the DMA itself; from
        slot 9 the mandatory ring wait takes the slot, so every
        cross-engine dep is moved to a Pool nop shim emitted just
        before. WAW edges against earlier SWDGE DMAs are ordered by the
        qPoolDynamic FIFO and removed."""
        nonlocal n_swdge
        shim_nop = emit("pool", nc.gpsimd.nop()) if n_swdge >= 8 else None
        dma = emit("pool", emit_fn())
        # Strip Tile's WAW sync edges against earlier SWDGE DMAs (the FIFO
        # orders them) -- but keep the nosync stream-pin edge emit() added.
        pin = swdge_fifo[-1].ins.name if (swdge_fifo and shim_nop is None) else None
        for prior in swdge_fifo:
            if prior.ins.name != pin:
                dma.ins.try_remove_dependency(prior.ins.name)
            elif prior.ins.name in set(dma.ins.sync_dependency_names()):
                # sync WAW edge exists alongside the pin; demote it: remove
                # both, then re-add the nosync pin.
                dma.ins.try_remove_dependency(prior.ins.name)
                add_dep_helper(dma.ins, prior.ins, False, "fifo order repin")
        if shim_nop is not None:
            for dep_name in list(dma.ins.sync_dependency_names()):
                dep = created.get(dep_name)
                if dep is not None:
                    add_dep_helper(shim_nop.ins, dep, True, "swdge shim wait")
                    dma.ins.try_remove_dependency(dep_name)
        swdge_fifo.append(dma)
        n_swdge += 1
        return dma

    with tile.TileContext(nc) as tc:
        with (
            tc.tile_pool(name="xpool", bufs=xbufs) as xpool,
            tc.tile_pool(name="spool", bufs=2) as spool,
            tc.tile_pool(name="ipool", bufs=n_tiles) as ipool,
            tc.tile_pool(name="gpool", bufs=gbufs) as gpool,
            tc.tile_pool(name="rpool", bufs=1) as rpool,
            tc.tile_pool(name="opool", bufs=n_tiles) as opool,
        ):
            xts = [xpool.tile([P, q], F32, name="xt", tag="xt") for _ in range(xbufs)]
            gts = [
                gpool.tile([P, TOPK * e], BF16, name="g8", tag="g8")
                for _ in range(gbufs)
            ]

            t1 = rpool.tile([P, 4 * e], BF16, name="t1")
            t2 = rpool.tile([P, 2 * e], F32, name="t2")
            xls = [None] * n_tiles
            idx8s = [None] * n_tiles
            maxidxs = [None] * n_tiles
            gathers = [None] * n_tiles
            prev_adds = []


            def emit_xload(i):
                xt = xts[i % xbufs]
                dma = emit("sp", nc.sync.dma_start(xt[:], x_d[i * P : (i + 1) * P, :]))
                if i >= xbufs:
                    # The WAR on the old tile's readers (max8/maxidx) is the
                    # one allowed wait; the WAW on the old x-load is implied
                    # by it (those readers observed that write) -- drop it.
                    dma.ins.try_remove_dependency(xls[i - xbufs].ins.name)
                xls[i] = dma

            def emit_topk(i):
                xt = xts[i % xbufs]
                s8 = spool.tile([P, TOPK], F32, name="s8", tag="s8")
                emit("dve", nc.vector.max(out=s8[:], in_=xt[:]))
                idx8 = ipool.tile([P, TOPK], U32, name="idx8", tag="idx8")
                maxidxs[i] = emit(
                    "dve",
                    nc.vector.max_index(out=idx8[:], in_max=s8[:], in_values=xt[:]),
                )
                idx8s[i] = idx8

            def emit_gather(i):
                # One indirect DMA per j: the HW indirect DMA consumes ONE
                # offset per partition (a multi-column offset AP silently
                # degenerates to "first index + consecutive rows").
                g = gts[i % gbufs]
                gathers[i] = []
                for j in range(TOPK):
                    dma = swdge_emit(
                        lambda j=j: nc.gpsimd.indirect_dma_start(
                            out=g[:, j * e : (j + 1) * e],
                            out_offset=None,
                            in_=wt_d[:],
                            in_offset=bass.IndirectOffsetOnAxis(
                                ap=idx8s[i][:, j : j + 1], axis=0
                            ),
                        )
                    )
                    gathers[i].append(dma)

            def emit_reduce_store(i):
                g = gts[i % gbufs]
                # The 8 gathers complete on 8 different DMASW lane sems; one
                # instruction can wait only one. Spread 7 of the waits over
                # cheap DVE nops; pass1 keeps the last gather's wait.
                for gdma in gathers[i][:-1]:
                    nop = emit("dve", nc.vector.nop())
                    add_dep_helper(nop.ins, gdma.ins, True, "gather wait")
                t1 = rpool.tile([P, 4 * e], BF16, name="t1", tag="t1")
                p1 = emit("dve", nc.vector.tensor_add(
                    t1[:], g[:, 0 : 4 * e], g[:, 4 * e : 8 * e]
                ))
                for gdma in gathers[i][:-1]:
                    p1.ins.try_remove_dependency(gdma.ins.name)
                t2 = rpool.tile([P, 2 * e], F32, name="t2", tag="t2")
                emit("dve", nc.vector.tensor_add(
                    t2[:], t1[:, 0 : 2 * e], t1[:, 2 * e : 4 * e]
                ))
                o = opool.tile([P, e], F32, name="o", tag="o")
                emit("dve", nc.vector.tensor_add(o[:], t2[:, 0:e], t2[:, e : 2 * e]))
                # placeholder
                swdge_emit(
                    lambda: nc.gpsimd.dma_start(out_d[i * P : (i + 1) * P, :], o[:])
                )

            # Software-pipelined emission: DVE runs topk(i+1) before
            # reduce(i) so the gather latency of tile i hides behind the
            # next tile's topk scan.
            for i in range(n_tiles):
                emit_xload(i)
                emit_topk(i)
                emit_gather(i)
                if i >= 1:
                    emit_reduce_store(i - 1)
            emit_reduce_store(n_tiles - 1)

            # Quiesce: single-wait SP nops observing every DMA lane's final
            # tick so the kernel-tail drains don't need multi-wait encodings.
            tail = xls + swdge_fifo[-8:] + [maxidxs[-1]] + prev_adds
            for tgt in tail:
                n = emit("sp", nc.sync.nop())
                add_dep_helper(n.ins, tgt.ins, True, "tail quiesce")

    return nc


def _prep_wt(W: np.ndarray) -> np.ndarray:
    """W [e, q] f32 -> WT [q, e] bf16 contiguous."""
    return np.ascontiguousarray(W.T).astype(ml_dtypes.bfloat16)


_CACHED = {}


def _get_nc():
    if "nc" not in _CACHED:
        _CACHED["nc"] = build_bass()
    return _CACHED["nc"]


def _in_maps(x: np.ndarray, W: np.ndarray) -> list[dict]:
    xf = np.asarray(x, dtype=np.float32).reshape(T_TOTAL, Q)
    WT = _prep_wt(np.asarray(W, dtype=np.float32))
    return [
        {
            "x": np.ascontiguousarray(xf[c * T_CORE : (c + 1) * T_CORE]),
            "wt": WT,
        }
        for c in range(N_CORES)
    ]


def kernel(x: np.ndarray, W: np.ndarray) -> np.ndarray:
    x = np.asarray(x, dtype=np.float32)
    W = np.asarray(W, dtype=np.float32)
    assert x.shape == (B, S, Q) and W.shape == (E, Q)

    nc = _get_nc()
    in_maps = _in_maps(x, W)
    res = run_bass_kernel_spmd(nc, in_maps, core_ids=list(range(N_CORES)))
    out = np.concatenate([r["out"] for r in res.results], axis=0)
    return np.ascontiguousarray(out.reshape(B, S, E).astype(np.float32))


# revision 27
# speedup vs baseline: 1.0094x; 1.0094x over previous
"""Trainium2 Bass kernel: NKQuantizer2 top-k masking (k=8).

reference:  kh = topk_hot(x, 8)          # [B,S,Q] 0/1 mask, top-8 per token
            out = einsum('bsq,eq->bse', kh, W)

Per token: out[t] = sum_{q in top8(x[t])} W[:, q] -- an 8-way embedding
gather-sum from W.T [Q, E].

Strategy (data-parallel over tokens across 8 cores, W.T bf16 in HBM):
  Per 128-token tile on each core:
    1. DMA x tile [128, 8192] f32 HBM->SBUF (HWDGE, 8 loads = the 8 HW
       lanes, one wait each)
    2. DVE Max8 -> top-8 values per token; DVE MaxIndex -> their indices
       (exact, ties -> first occurrence, matching jax.lax.top_k)
    3. ONE batched indirect-DMA gather per tile (SWDGE):
       g[p, j, :] = WT[idx8[p, j], :], 1024 descriptors in a single
       instruction -- amortizes the ~1us fixed SWDGE overhead that
       dominated when issued as 8 separate accumulate-gathers, and drops
       the CCE read-modify-write from the DMA datapath.
    4. DVE tree-reduce over j (bf16 pass, then f32) -> o [128, 512] f32
    5. Store o -> out rows, also on the SWDGE FIFO.

Toolchain constraints handled here:
  * Any instruction can encode at most ONE semaphore wait; waits on the
    same semaphore merge (max tick), so multiple deps are fine only if
    they land on one engine's semaphore.
  * 8 global HWDGE lanes + 8 SWDGE lanes; a DMA on a reused lane gets a
    mandatory ring wait injected by codegen, which uses up its one slot.
    So SWDGE ops 9..16 carry their cross-engine dep on a preceding Pool
    nop shim instead (the SWDGE descriptor generator executes waits in
    program order, so a nop wait gates the following descriptor).
  * The tile scheduler reorders instruction streams, which can turn
    same-engine deps into semaphore waits (and scrambles walrus's
    ring-lane assignment). Every instruction is chained to its
    same-engine predecessor with a non-sync edge to pin stream order.
"""

import numpy as np
import ml_dtypes

import concourse.bass as bass
import concourse.mybir as mybir
import concourse.tile as tile
from concourse.bass_utils import run_bass_kernel_spmd
from concourse.tile_rust import add_dep_helper

B, S, Q, E, TOPK = 4, 2048, 8192, 512, 8
N_CORES = 8
P = 128
T_TOTAL = B * S                 # 8192 tokens
T_CORE = T_TOTAL // N_CORES     # 1024 tokens per core

F32 = mybir.dt.float32
BF16 = mybir.dt.bfloat16
U32 = mybir.dt.uint32


def build_bass(t_core=T_CORE, q=Q, e=E):
    """Build the per-core Bass program (SPMD: same program on all cores)."""
    n_tiles = t_core // P
    xbufs = min(4, n_tiles)
    gbufs = min(3, n_tiles)

    nc = bass.Bass(trn_type="TRN2", target_bir_lowering=False)
    x_d = nc.dram_tensor("x", [t_core, q], F32, kind="ExternalInput")
    wt_d = nc.dram_tensor("wt", [q, e], BF16, kind="ExternalInput")
    out_d = nc.dram_tensor("out", [t_core, e], F32, kind="ExternalOutput")

    created = {}         # name -> mybir instruction, everything we emit
    stream_last = {}     # engine-stream key -> last instruction (pinning)

    def emit(key, bass_ins):
        """Register an instruction and chain it into its engine stream."""
        ins = bass_ins.ins
        if key in stream_last:
            add_dep_helper(ins, stream_last[key], False, f"{key} order")
        stream_last[key] = ins
        created[ins.name] = ins
        return bass_ins

    n_swdge = 0          # SWDGE FIFO slot counter (8 lanes before reuse)
    swdge_fifo = []      # all SWDGE DMAs in program order

    def swdge_emit(emit_fn):
        """Emit a SWDGE DMA. In the first 8 FIFO slots its (merged,
        single-semaphore) cross-engine deps ride on the DMA itself; from
        slot 9 the mandatory ring wait takes the slot, so every
        cross-engine dep is moved to a Pool nop shim emitted just
        before. WAW edges against earlier SWDGE DMAs are ordered by the
        qPoolDynamic FIFO and removed."""
        nonlocal n_swdge
        shim_nop = emit("pool", nc.gpsimd.nop()) if n_swdge >= 8 else None
        dma = emit("pool", emit_fn())
        # Strip Tile's WAW sync edges against earlier SWDGE DMAs (the FIFO
        # orders them) -- but keep the nosync stream-pin edge emit() added.
        pin = swdge_fifo[-1].ins.name if (swdge_fifo and shim_nop is None) else None
        for prior in swdge_fifo:
            if prior.ins.name != pin:
                dma.ins.try_remove_dependency(prior.ins.name)
            elif prior.ins.name in set(dma.ins.sync_dependency_names()):
                # sync WAW edge exists alongside the pin; demote it: remove
                # both, then re-add the nosync pin.
                dma.ins.try_remove_dependency(prior.ins.name)
                add_dep_helper(dma.ins, prior.ins, False, "fifo order repin")
        if shim_nop is not None:
            for dep_name in list(dma.ins.sync_dependency_names()):
                dep = created.get(dep_name)
                if dep is not None:
                    add_dep_helper(shim_nop.ins, dep, True, "swdge shim wait")
                    dma.ins.try_remove_dependency(dep_name)
        swdge_fifo.append(dma)
        n_swdge += 1
        return dma

    with tile.TileContext(nc) as tc:
        with (
            tc.tile_pool(name="xpool", bufs=xbufs) as xpool,
            tc.tile_pool(name="spool", bufs=2) as spool,
            tc.tile_pool(name="ipool", bufs=n_tiles) as ipool,
            tc.tile_pool(name="gpool", bufs=gbufs) as gpool,
            tc.tile_pool(name="rpool", bufs=1) as rpool,
            tc.tile_pool(name="opool", bufs=n_tiles) as opool,
        ):
            xts = [xpool.tile([P, q], F32, name="xt", tag="xt") for _ in range(xbufs)]
            gts = [
                gpool.tile([P, TOPK * e], BF16, name="g8", tag="g8")
                for _ in range(gbufs)
            ]

            t1 = rpool.tile([P, 4 * e], BF16, name="t1")
            t2 = rpool.tile([P, 2 * e], F32, name="t2")
            xls = [None] * n_tiles
            idx8s = [None] * n_tiles
            maxidxs = [None] * n_tiles
            gathers = [None] * n_tiles
            prev_adds = []


            def emit_xload(i):
                xt = xts[i % xbufs]
                dma = emit("sp", nc.sync.dma_start(xt[:], x_d[i * P : (i + 1) * P, :]))
                if i >= xbufs:
                    # The WAR on the old tile's readers (max8/maxidx) is the
                    # one allowed wait; the WAW on the old x-load is implied
                    # by it (those readers observed that write) -- drop it.
                    dma.ins.try_remove_dependency(xls[i - xbufs].ins.name)
                xls[i] = dma

            def emit_topk(i):
                xt = xts[i % xbufs]
                s8 = spool.tile([P, TOPK], F32, name="s8", tag="s8")
                emit("dve", nc.vector.max(out=s8[:], in_=xt[:]))
                idx8 = ipool.tile([P, TOPK], U32, name="idx8", tag="idx8")
                maxidxs[i] = emit(
                    "dve",
                    nc.vector.max_index(out=idx8[:], in_max=s8[:], in_values=xt[:]),
                )
                idx8s[i] = idx8

            def emit_gather(i):
                # One indirect DMA per j: the HW indirect DMA consumes ONE
                # offset per partition (a multi-column offset AP silently
                # degenerates to "first index + consecutive rows").
                g = gts[i % gbufs]
                gathers[i] = []
                for j in range(TOPK):
                    dma = swdge_emit(
                        lambda j=j: nc.gpsimd.indirect_dma_start(
                            out=g[:, j * e : (j + 1) * e],
                            out_offset=None,
                            in_=wt_d[:],
                            in_offset=bass.IndirectOffsetOnAxis(
                                ap=idx8s[i][:, j : j + 1], axis=0
                            ),
                        )
                    )
                    gathers[i].append(dma)

            def emit_reduce_store(i):
                # Tree-reduce on the Pool engine (TensorTensor is valid under
                # the default 'proxy' gpsimd library) -- frees ~25us of DVE,
                # the critical path. The 8 gathers complete on 8 DMASW lane
                # sems; one instruction can wait only one, so 7 waits ride
                # Pool nops and pass1 keeps the last gather's. Pool-internal
                # t1/t2 reuse is ordered by the serial engine; those deps are
                # stripped so each add keeps a single wait.
                g = gts[i % gbufs]
                for gdma in gathers[i][:-1]:
                    nop = emit("pool", nc.gpsimd.nop())
                    add_dep_helper(nop.ins, gdma.ins, True, "gather wait")
                adds = []
                p1 = emit("pool", nc.gpsimd.tensor_add(
                    t1[:], g[:, 0 : 4 * e], g[:, 4 * e : 8 * e]
                ))
                adds.append(p1)
                for gdma in gathers[i][:-1]:
                    p1.ins.try_remove_dependency(gdma.ins.name)
                adds.append(emit("pool", nc.gpsimd.tensor_add(
                    t2[:], t1[:, 0 : 2 * e], t1[:, 2 * e : 4 * e]
                )))
                o = opool.tile([P, e], F32, name="o", tag="o")
                adds.append(emit("pool", nc.gpsimd.tensor_add(
                    o[:], t2[:, 0:e], t2[:, e : 2 * e]
                )))
                for a in adds:
                    for pa in prev_adds:
                        a.ins.try_remove_dependency(pa.ins.name)
                prev_adds.clear()
                prev_adds.extend(adds)
                swdge_emit(
                    lambda: nc.gpsimd.dma_start(out_d[i * P : (i + 1) * P, :], o[:])
                )

            # Software-pipelined emission: DVE runs topk(i+1) before
            # reduce(i) so the gather latency of tile i hides behind the
            # next tile's topk scan.
            for i in range(n_tiles):
                emit_xload(i)
                emit_topk(i)
                emit_gather(i)
                if i >= 1:
                    emit_reduce_store(i - 1)
            emit_reduce_store(n_tiles - 1)

            # Quiesce: single-wait SP nops observing every DMA lane's final
            # tick so the kernel-tail drains don't need multi-wait encodings.
            tail = xls + swdge_fifo[-8:] + [maxidxs[-1]] + prev_adds
            for tgt in tail:
                n = emit("sp", nc.sync.nop())
                add_dep_helper(n.ins, tgt.ins, True, "tail quiesce")

    return nc


def _prep_wt(W: np.ndarray) -> np.ndarray:
    """W [e, q] f32 -> WT [q, e] bf16 contiguous."""
    return np.ascontiguousarray(W.T).astype(ml_dtypes.bfloat16)


_CACHED = {}


def _get_nc():
    if "nc" not in _CACHED:
        _CACHED["nc"] = build_bass()
    return _CACHED["nc"]


def _in_maps(x: np.ndarray, W: np.ndarray) -> list[dict]:
    xf = np.asarray(x, dtype=np.float32).reshape(T_TOTAL, Q)
    WT = _prep_wt(np.asarray(W, dtype=np.float32))
    return [
        {
            "x": np.ascontiguousarray(xf[c * T_CORE : (c + 1) * T_CORE]),
            "wt": WT,
        }
        for c in range(N_CORES)
    ]


def kernel(x: np.ndarray, W: np.ndarray) -> np.ndarray:
    x = np.asarray(x, dtype=np.float32)
    W = np.asarray(W, dtype=np.float32)
    assert x.shape == (B, S, Q) and W.shape == (E, Q)

    nc = _get_nc()
    in_maps = _in_maps(x, W)
    res = run_bass_kernel_spmd(nc, in_maps, core_ids=list(range(N_CORES)))
    out = np.concatenate([r["out"] for r in res.results], axis=0)
    return np.ascontiguousarray(out.reshape(B, S, E).astype(np.float32))
